# revision 13
# baseline (speedup 1.0000x reference)
"""Trainium2 Bass kernel: GQA attention layer (RoPE + causal attention + projections).

Strategy (8 NeuronCores, tensor-parallel by head):
  - Each core owns 2 query heads + 1 kv head (NH=16, NKV=8 -> GQA pairs align
    with cores exactly). QKV projection, RoPE, and attention for those heads run
    fully locally -- zero K/V communication.
  - q1/q2/k projections run in fp8 (e4m3) with DoubleRow perf mode (256-deep
    contraction per instruction, 2 fp8 mults/cell) -- safe because scores are
    tiny and softmax normalization cancels quantization noise. The v projection
    and everything downstream stays bf16 (attention output is a near-uniform
    mean over tokens, so independent operand noise passes through at full
    relative strength -- fp8 there would blow the error budget).
  - Activations/weights are scaled by powers of two host-side so fp8 operands
    sit in e4m3's sweet spot; q/k stay scaled through RoPE and QK^T and the
    combined 2^-32 descale folds into the Exp activation's input scale (exact).
  - Attention is computed in the S^T orientation ([keys, q]); score chunks are
    computed in pairs into a [128,1024] PSUM tile so one Exp activation covers
    two chunks; softmax denominator accumulates via an all-ones stationary
    matmul over 4-chunk partial sums (DVE pre-adds).
  - DMAs are spread over three queues (sync HW, scalar HW, gpsimd SW) so
    weight/const loads run parallel to the activation stream and phase-C input
    loads are not head-of-line blocked behind attention stores.
  - After attention, one AllToAll per batch reshards activations from
    head-sharded to token-sharded; each core then runs o_proj for its 512
    tokens (bf16, fp32 PSUM) and emits bf16 outputs (host casts to fp32).
"""

import os
from contextlib import ExitStack

import ml_dtypes
import numpy as np

import concourse.bass as bass
import concourse.tile as tile
from concourse import bacc, mybir
from concourse.bass_utils import run_bass_kernel_spmd

# Problem shapes (hardcoded per spec nn_AvaAttention_36249523978775).
B, T, HID = 2, 2048, 2048
NH, NKV, HD = 16, 8, 128
SCALE = HD ** -0.5
NC = 8
TT = B * T  # 4096 flat tokens, b-major
NEG = -3.0e38  # large-negative additive mask in the 2^32-scaled score domain

F32 = mybir.dt.float32
BF = mybir.dt.bfloat16
F8 = mybir.dt.float8e4

NPBF = ml_dtypes.bfloat16
NPF8 = ml_dtypes.float8_e4m3

TN = 512           # token chunk for projection moving operand
NG = TT // TN      # 8 projection token groups
NHC = HID // 128   # 16 contraction chunks
NQC = T // 256     # 8 query strips of 256 per batch
NKC = T // 128     # 16 key chunks of 128 per batch

# power-of-two fp8 scaling (descale 2^-32 folds into the Exp input scale)
A_H = 64.0         # hidden activations
A_Q = 4096.0       # q weights (on top of the folded SCALE)
A_K = 256.0        # k weights
EXP_SCALE = 1.0 / (A_H * A_H * A_Q * A_K)  # == 2^-32

_CACHE = {}
last_results = None  # test harness reads exec_time_ns from here


def _build(mode: str):
    """Build the SPMD graph. mode in {"causal", "none", "generic"}."""
    nc = bacc.Bacc("TRN2", target_bir_lowering=False, debug=False, num_devices=NC)

    hT8_e = nc.declare_dram_parameter("hT8", [NG, NHC, 128, TN], F8, isOutput=False)
    hTv_e = nc.declare_dram_parameter("hTv", [NG, NHC, 128, TN], BF, isOutput=False)
    w8_e = nc.declare_dram_parameter("w8", [128, 3, NHC, 128], F8, isOutput=False)
    wv_e = nc.declare_dram_parameter("wv", [128, NHC, 128], BF, isOutput=False)
    woT_e = nc.declare_dram_parameter("woT", [NH * HD, HID], BF, isOutput=False)
    ropeC_e = nc.declare_dram_parameter("ropeC", [128, T], BF, isOutput=False)
    ropeS_e = nc.declare_dram_parameter("ropeS", [128, T], BF, isOutput=False)
    ones_e = nc.declare_dram_parameter("ones", [128, 128], BF, isOutput=False)
    ident_e = nc.declare_dram_parameter("ident", [128, 128], BF, isOutput=False)
    pat_e = None
    maskT_e = None
    if mode == "causal":
        # [key(128), (sub, head, 256q)] additive mask for the two diagonal
        # chunks of a strip, flattened to match the paired score tile layout.
        pat_e = nc.declare_dram_parameter("pat", [128, 1024], F32, isOutput=False)
    elif mode == "generic":
        maskT_e = nc.declare_dram_parameter("maskT", [T, T], F32, isOutput=False)
    out_e = nc.declare_dram_parameter("out", [512, HID], BF, isOutput=True)

    with tile.TileContext(nc) as tc:
        with tc.tile_pool(name="consts", bufs=1) as consts, \
             tc.tile_pool(name="dram", bufs=1, space="DRAM") as dram:

            ones_t = consts.tile([128, 128], BF)
            ident_t = consts.tile([128, 128], BF)
            pat_t = None
            if mode == "causal":
                pat_t = consts.tile([128, 1024], F32)

            a2a_in = [dram.tile([NC, 256, 256], BF, name=f"a2a_in{b}") for b in range(B)]
            a2a_out = [dram.tile([NC, 256, 256], BF, name=f"a2a_out{b}") for b in range(B)]

            # o_proj weights: tiles reserved early (pool-nesting order), DMA
            # emitted mid-stream on the gpsimd queue in 1MB chunks.
            es_wo = ExitStack()
            wop = es_wo.enter_context(tc.tile_pool(name="wop", bufs=1))
            wo_res = [wop.tile([128, NH, 1024], BF, name=f"wo{half}")
                      for half in range(2)]

            es = ExitStack()
            big = es.enter_context(tc.tile_pool(name="big", bufs=1))
            # Persistent activations (my heads, all tokens). q/k are scaled
            # (2^18 / 2^14); v is true-valued.
            q_sb = big.tile([128, 2, TT], BF)      # Q^T, 2 q heads
            k_sb = big.tile([128, TT], BF)         # K^T, 1 kv head
            v_sb = big.tile([128, TT // 128, 128], BF)  # V natural, [tok-chunk, d]

            # -------- Phase A+B interleaved: projection feeds attention ------
            with tc.tile_pool(name="wrope", bufs=1) as wrope, \
                 tc.tile_pool(name="ht8", bufs=6) as ht8_pool, \
                 tc.tile_pool(name="htv", bufs=6) as htv_pool, \
                 tc.tile_pool(name="psA", bufs=2, space="PSUM") as psA, \
                 tc.tile_pool(name="psS", bufs=2, space="PSUM") as psS, \
                 tc.tile_pool(name="ropetmp", bufs=3) as rtmp, \
                 tc.tile_pool(name="vtmp", bufs=2) as vtmp, \
                 tc.tile_pool(name="psPV", bufs=1, space="PSUM") as psPV, \
                 tc.tile_pool(name="psDen", bufs=1, space="PSUM") as psDen, \
                 tc.tile_pool(name="pt", bufs=3) as pt_pool, \
                 tc.tile_pool(name="attev", bufs=2) as attev, \
                 tc.tile_pool(name="mt", bufs=4) as mt_pool:
                ropeC_t = wrope.tile([128, T], BF)
                ropeS_t = wrope.tile([128, T], BF)
                w8_t = wrope.tile([128, 3, NHC, 128], F8)
                wv_t = wrope.tile([128, NHC, 128], BF)
                # Weight/const loads on the scalar HW queue run parallel to
                # the sync-queue activation stream: first matmul needs only
                # w8 stream 0 + the first ht8 unit.
                for s in range(3):
                    nc.scalar.dma_start(w8_t[:, s], w8_e[:, s])
                nc.scalar.dma_start(wv_t[:], wv_e[:])
                nc.scalar.dma_start(ident_t[:], ident_e[:])
                nc.scalar.dma_start(ropeC_t[:], ropeC_e[:])
                nc.scalar.dma_start(ropeS_t[:], ropeS_e[:])
                nc.scalar.dma_start(ones_t[:], ones_e[:])
                if mode == "causal":
                    nc.scalar.dma_start(pat_t[:], pat_e[:])

                def attention_strip(b, qc):
                    cmax = 2 * qc + 2 if mode == "causal" else NKC
                    ntile = cmax // 2
                    mv = q_sb[:, :, b * T + 256 * qc: b * T + 256 * qc + 256]
                    pv = psPV.tile([128, 512], F32, name="pv", tag="pv")
                    den = psDen.tile([128, 512], F32, name="den", tag="den")
                    pts_prev = None
                    den_started = False
                    for ti in range(ntile):
                        c0 = 2 * ti
                        st2 = psS.tile([128, 1024], F32, name="st2", tag="st2")
                        nc.tensor.matmul(
                            st2[:, 0:512],
                            k_sb[:, b * T + 128 * c0: b * T + 128 * c0 + 128],
                            mv, start=True, stop=True)
                        nc.tensor.matmul(
                            st2[:, 512:1024],
                            k_sb[:, b * T + 128 * c0 + 128: b * T + 128 * c0 + 256],
                            mv, start=True, stop=True)
                        if mode == "causal" and ti == ntile - 1:
                            # both diagonal chunks masked by one 1024-wide add
                            nc.vector.tensor_add(st2[:], st2[:], pat_t[:])
                        elif mode == "generic":
                            for half in range(2):
                                mt = mt_pool.tile([128, 256], F32, name="mt", tag="mt")
                                nc.scalar.dma_start(
                                    mt[:], maskT_e[128 * (c0 + half):128 * (c0 + half) + 128,
                                                   256 * qc:256 * qc + 256])
                                o = 512 * half
                                nc.vector.tensor_add(st2[:, o:o + 256], st2[:, o:o + 256], mt[:])
                                nc.vector.tensor_add(st2[:, o + 256:o + 512], st2[:, o + 256:o + 512], mt[:])
                        pt2 = pt_pool.tile([128, 1024], BF, name="pt2", tag="pt2")
                        nc.scalar.activation(pt2[:], st2[:],
                                             mybir.ActivationFunctionType.Exp,
                                             scale=EXP_SCALE)
                        nc.tensor.matmul(pv[:], v_sb[:, NKC * b + c0, :], pt2[:, 0:512],
                                         start=(ti == 0), stop=False)
                        nc.tensor.matmul(pv[:], v_sb[:, NKC * b + c0 + 1, :], pt2[:, 512:1024],
                                         start=False, stop=(ti == ntile - 1))
                        # denominator: DVE pre-sums 4 chunks per ones-matvec
                        pts = pt_pool.tile([128, 512], BF, name="pts", tag="pts")
                        nc.vector.tensor_add(pts[:], pt2[:, 0:512], pt2[:, 512:1024])
                        last = ti == ntile - 1
                        if ti % 2 == 0 and not last:
                            pts_prev = pts
                        else:
                            if ti % 2 == 1:
                                ptm = pt_pool.tile([128, 512], BF, name="ptm", tag="pts")
                                nc.vector.tensor_add(ptm[:], pts_prev[:], pts[:])
                            else:
                                ptm = pts
                            nc.tensor.matmul(den[:], ones_t[:], ptm[:],
                                             start=not den_started, stop=last)
                            den_started = True
                    # den rows are all identical (ones stationary) == softmax denom
                    den_rb = attev.tile([128, 512], F32, name="den_rb", tag="den_rb")
                    nc.vector.reciprocal_approx_fast(den_rb[:], den[:])
                    ao = attev.tile([128, 512], BF, name="ao", tag="ao")
                    nc.vector.tensor_mul(ao[:], pv[:], den_rb[:])
                    # ao on gpsimd: on the scalar queue this descriptor (which
                    # waits on the whole strip pipeline) head-of-line blocks
                    # the next strip's exps and delays the AllToAll trigger
                    nc.gpsimd.dma_start(
                        a2a_in[b][qc].rearrange("(h p) t -> p h t", p=128),
                        ao[:].rearrange("p (h t) -> p h t", h=2))

                for g in range(NG):
                    t0 = g * TN
                    # chunk-pair slices for the fp8 DoubleRow moving operand;
                    # group 0 uses small units so the first matmuls start as
                    # early as possible (DMA-paced kernel head)
                    ht8_pairs = []
                    htvs = []
                    if g == 0:
                        for j in range(NHC // 4):
                            ht8 = ht8_pool.tile([128, 4, TN], F8, name="ht8", tag="ht8")
                            for h2 in range(2):
                                nc.sync.dma_start(
                                    ht8[:, 2 * h2:2 * h2 + 2, :],
                                    hT8_e[g, 4 * j + 2 * h2:4 * j + 2 * h2 + 2]
                                    .rearrange("c p t -> p c t"))
                                ht8_pairs.append(ht8[:, 2 * h2:2 * h2 + 2, :])
                    else:
                        for j in range(NHC // 4):
                            ht8 = ht8_pool.tile([128, 4, TN], F8, name="ht8", tag="ht8")
                            nc.sync.dma_start(ht8[:], hT8_e[g, 4 * j:4 * j + 4].rearrange("c p t -> p c t"))
                            ht8_pairs.append(ht8[:, 0:2, :])
                            ht8_pairs.append(ht8[:, 2:4, :])
                    for j in range(NHC // 4):
                        htv = htv_pool.tile([128, 4, TN], BF, name="htv", tag="htv")
                        nc.gpsimd.dma_start(htv[:], hTv_e[g, 4 * j:4 * j + 4].rearrange("c p t -> p c t"))
                        htvs.append(htv)
                    # o_proj weights spread over the scalar queue mid-stream
                    if g in (1, 2, 3, 4):
                        half = (g - 1) // 2
                        for hc in (0, 1):
                            hco = 2 * ((g - 1) % 2) + hc
                            nc.scalar.dma_start(
                                wo_res[half][:, 4 * hco:4 * hco + 4, :],
                                woT_e[512 * hco:512 * (hco + 1),
                                      half * 1024:(half + 1) * 1024]
                                .rearrange("(h p) n -> p h n", p=128))
                    ctab = g % (T // TN) * TN  # rope table column offset
                    # q1/q2/k: fp8 DoubleRow, 256-deep contraction per matmul
                    for s in range(3):
                        ps = psA.tile([128, TN], F32, name="psA", tag="psA")
                        for j in range(NHC // 2):
                            nc.tensor.matmul(ps[:], w8_t[:, s, 2 * j:2 * j + 2, :],
                                             ht8_pairs[j],
                                             start=(j == 0), stop=(j == NHC // 2 - 1),
                                             perf_mode=mybir.MatmulPerfMode.DoubleRow)
                        # RoPE: out = ps*C + rot(ps)*S  (S carries the sign)
                        if s < 2:
                            dst = q_sb[:, s, t0:t0 + TN]
                        else:
                            dst = k_sb[:, t0:t0 + TN]
                        csl = ropeC_t[:, ctab:ctab + TN]
                        ssl = ropeS_t[:, ctab:ctab + TN]
                        t1 = rtmp.tile([128, TN], BF, name="t1", tag="t1")
                        t2 = rtmp.tile([128, TN], BF, name="t2", tag="t2")
                        nc.vector.tensor_mul(t1[:], ps[:], csl)
                        nc.vector.tensor_mul(t2[0:64, :], ps[64:128, :], ssl[0:64, :])
                        nc.vector.tensor_mul(t2[64:128, :], ps[0:64, :], ssl[64:128, :])
                        nc.vector.tensor_add(dst, t1[:], t2[:])
                    # v: bf16, V^T -> transpose to V natural via PE
                    psv = psA.tile([128, TN], F32, name="psv", tag="psA")
                    for hc in range(NHC):
                        nc.tensor.matmul(psv[:], wv_t[:, hc, :],
                                         htvs[hc // 4][:, hc % 4, :],
                                         start=(hc == 0), stop=(hc == NHC - 1))
                    vt = vtmp.tile([128, TN], BF, name="vt", tag="vt")
                    nc.scalar.copy(vt[:], psv[:])
                    for jj in range(TN // 128):
                        trp = psS.tile([128, 1024], F32, name="trp", tag="st2")
                        trp_bf = trp.bitcast(BF)[:, 0:128]
                        nc.tensor.transpose(trp_bf, vt[:, jj * 128:(jj + 1) * 128], ident_t[:])
                        if jj % 2 == 0:
                            nc.vector.tensor_copy(v_sb[:, g * (TN // 128) + jj, :], trp_bf)
                        else:
                            nc.scalar.copy(v_sb[:, g * (TN // 128) + jj, :], trp_bf)
                    # attention strips unlocked by this group
                    if mode == "causal":
                        b = g // 4
                        strips = [(b, 2 * (g % 4)), (b, 2 * (g % 4) + 1)]
                    else:
                        # non-causal strips read every key chunk of the batch
                        strips = ([(g // 4, qc) for qc in range(NQC)]
                                  if g in (3, 7) else [])
                    for b, qc in strips:
                        attention_strip(b, qc)
                    if g in (3, 7):
                        nc.gpsimd.collective_compute(
                            "AllToAll", mybir.AluOpType.bypass,
                            replica_groups=[list(range(NC))],
                            ins=[a2a_in[g // 4][:].opt()],
                            outs=[a2a_out[g // 4][:].opt()])

            es.close()  # free q/k/v SBUF before o_proj

            # ---------------- Phase C: o_proj --------------------------------
            with tc.tile_pool(name="attg", bufs=2) as attg_pool, \
                 tc.tile_pool(name="psF", bufs=2, space="PSUM") as psF, \
                 tc.tile_pool(name="fo", bufs=2) as fo_pool:
                for p in range(B):
                    att_g = attg_pool.tile([128, NH, 256], BF, name="attg", tag="attg")
                    for j in range(NC):
                        nc.sync.dma_start(
                            att_g[:, 2 * j:2 * j + 2, :],
                            a2a_out[p][j].rearrange("(h p) t -> p h t", p=128))
                    fins = [psF.tile([128, HID], F32, name="fin", tag="fin") for _ in range(2)]
                    # tch-outer so fins[0] completes early and its copies/DMA
                    # overlap fins[1]'s matmuls (shorter serial tail)
                    for tch in range(2):
                        for half in range(2):
                            for h in range(NH):
                                for n2 in range(2):
                                    nc.tensor.matmul(
                                        fins[tch][:, half * 1024 + n2 * 512: half * 1024 + (n2 + 1) * 512],
                                        att_g[:, h, tch * 128:(tch + 1) * 128],
                                        wo_res[half][:, h, n2 * 512:(n2 + 1) * 512],
                                        start=(h == 0), stop=(h == NH - 1))
                        fo = fo_pool.tile([128, HID], BF, name="fo", tag="fo")
                        for seg in range(4):
                            sl = slice(512 * seg, 512 * (seg + 1))
                            if seg % 2 == 0:
                                nc.vector.tensor_copy(fo[:, sl], fins[tch][:, sl])
                            else:
                                nc.scalar.copy(fo[:, sl], fins[tch][:, sl])
                        nc.sync.dma_start(
                            out_e[p * 256 + tch * 128: p * 256 + (tch + 1) * 128, :], fo[:])
            es_wo.close()

    nc.compile()
    return nc


def _host_prep(hidden_states, freqs_cos, freqs_sin, mask, w_qkv, w_o, kv_write_indices):
    idx = np.asarray(kv_write_indices).astype(np.int64)
    if not np.array_equal(idx, np.arange(T, dtype=np.int64)):
        raise NotImplementedError("kernel specialized for kv_write_indices == arange(T)")

    hs = np.asarray(hidden_states, dtype=np.float32).reshape(TT, HID)
    # [HID, TT] -> tiled [NG, NHC, 128, TN] so each DMA slice is contiguous
    hT = np.ascontiguousarray(
        hs.T.reshape(NHC, 128, NG, TN).transpose(2, 0, 1, 3))
    hTv = hT.astype(NPBF)
    hT8 = (hT * A_H).astype(NPF8)

    m2 = np.asarray(mask, dtype=np.float32).reshape(T, T)
    tril = np.tril(np.ones((T, T), dtype=bool))
    if not m2.any():
        mode = "none"
    elif (m2[tril] == 0).all() and (m2[~tril] <= -1e30).all():
        mode = "causal"
    else:
        mode = "generic"

    wq = np.asarray(w_qkv, dtype=np.float32)
    woT = np.ascontiguousarray(np.asarray(w_o, dtype=np.float32).T).astype(NPBF)

    def tile_w(wrows):
        # [128 out, HID] -> [NHC, 128 hid, 128 out] stationary tiles
        return np.ascontiguousarray(wrows.T).reshape(NHC, 128, 128)

    w8s, wvs = [], []
    for c in range(NC):
        q1 = wq[(2 * c) * HD:(2 * c + 1) * HD] * (SCALE * A_Q)
        q2 = wq[(2 * c + 1) * HD:(2 * c + 2) * HD] * (SCALE * A_Q)
        k = wq[NH * HD + c * HD: NH * HD + (c + 1) * HD] * A_K
        v = wq[(NH + NKV) * HD + c * HD: (NH + NKV) * HD + (c + 1) * HD]
        # [3, NHC, 128 hid, 128 out] -> [128 hid, 3, NHC, 128 out] (SBUF layout)
        w8s.append(np.ascontiguousarray(
            np.stack([tile_w(q1), tile_w(q2), tile_w(k)])
            .transpose(2, 0, 1, 3)).astype(NPF8))
        wvs.append(np.ascontiguousarray(
            tile_w(v).transpose(1, 0, 2)).astype(NPBF))

    cosT = np.asarray(freqs_cos, dtype=np.float32).T  # [64, T]
    sinT = np.asarray(freqs_sin, dtype=np.float32).T
    ropeC = np.ascontiguousarray(np.concatenate([cosT, cosT], axis=0)).astype(NPBF)
    ropeS = np.ascontiguousarray(np.concatenate([-sinT, sinT], axis=0)).astype(NPBF)

    consts = {
        "ropeC": ropeC,
        "ropeS": ropeS,
        "ones": np.ones((128, 128), NPBF),
        "ident": np.eye(128, dtype=np.float32).astype(NPBF),
    }
    if mode == "causal":
        # [key 128, (sub, head, 256 q)] for the strip-diagonal chunk pair
        kr = np.arange(128)[:, None]
        qr = np.arange(256)[None, :]
        pats = []
        for sub in range(2):
            p = np.where(kr + 128 * sub <= qr, np.float32(0.0), np.float32(NEG))
            pats.append(np.broadcast_to(p[:, None, :], (128, 2, 256)))
        pat = np.ascontiguousarray(
            np.concatenate(pats, axis=1).reshape(128, 1024)).astype(np.float32)
        consts["pat"] = pat
    elif mode == "generic":
        # mask values live in the 2^32-scaled score domain; clamp so the DVE
        # add cannot overflow fp32
        mscaled = np.maximum(m2.T.astype(np.float64) / EXP_SCALE, NEG)
        consts["maskT"] = np.ascontiguousarray(mscaled.astype(np.float32))

    in_maps = []
    for c in range(NC):
        m = {"hT8": hT8, "hTv": hTv, "w8": w8s[c], "wv": wvs[c], "woT": woT}
        m.update(consts)
        in_maps.append(m)
    return mode, in_maps


def kernel(hidden_states, freqs_cos, freqs_sin, k_cache, v_cache, mask, w_qkv,
           w_o, kv_write_indices):
    # k_cache/v_cache are fully overwritten (kv_write_indices == arange covers
    # every slot), so their incoming contents are irrelevant.
    global last_results
    mode, in_maps = _host_prep(hidden_states, freqs_cos, freqs_sin, mask,
                               w_qkv, w_o, kv_write_indices)
    if mode not in _CACHE:
        _CACHE[mode] = _build(mode)
    nc = _CACHE[mode]

    trace = bool(os.environ.get("BASS_KERNEL_TRACE"))
    res = run_bass_kernel_spmd(nc, in_maps, core_ids=list(range(NC)), trace=trace)
    last_results = res

    final = np.empty((B, T, HID), dtype=np.float32)
    for c in range(NC):
        o = np.asarray(res.results[c]["out"]).astype(np.float32)
        final[0, 256 * c:256 * (c + 1)] = o[0:256]
        final[1, 256 * c:256 * (c + 1)] = o[256:512]
    return final


# revision 17
# speedup vs baseline: 1.0436x; 1.0436x over previous
"""Trainium2 Bass kernel: GQA attention layer (RoPE + causal attention + projections).

Strategy (8 NeuronCores, tensor-parallel by head):
  - Each core owns 2 query heads + 1 kv head (NH=16, NKV=8 -> GQA pairs align
    with cores exactly). QKV projection, RoPE, and attention for those heads run
    fully locally -- zero K/V communication.
  - q1/q2/k projections run in fp8 (e4m3) with DoubleRow perf mode (256-deep
    contraction per instruction, 2 fp8 mults/cell) -- safe because scores are
    tiny and softmax normalization cancels quantization noise. The v projection
    and everything downstream stays bf16 (attention output is a near-uniform
    mean over tokens, so independent operand noise passes through at full
    relative strength -- fp8 there would blow the error budget).
  - Activations/weights are scaled by powers of two host-side so fp8 operands
    sit in e4m3's sweet spot; q/k stay scaled through RoPE and QK^T and the
    combined 2^-32 descale folds into the Exp activation's input scale (exact).
  - Attention is computed in the S^T orientation ([keys, q]); score chunks are
    computed in pairs into a [128,1024] PSUM tile so one Exp activation covers
    two chunks; softmax denominator accumulates via an all-ones stationary
    matmul over 4-chunk partial sums (DVE pre-adds).
  - DMAs are spread over three queues (sync HW, scalar HW, gpsimd SW) so
    weight/const loads run parallel to the activation stream and phase-C input
    loads are not head-of-line blocked behind attention stores.
  - After attention, one AllToAll per batch reshards activations from
    head-sharded to token-sharded; each core then runs o_proj for its 512
    tokens (bf16, fp32 PSUM) and emits bf16 outputs (host casts to fp32).
"""

import os
from contextlib import ExitStack

import ml_dtypes
import numpy as np

import concourse.bass as bass
import concourse.tile as tile
from concourse import bacc, mybir
from concourse.bass_utils import run_bass_kernel_spmd

# Problem shapes (hardcoded per spec nn_AvaAttention_36249523978775).
B, T, HID = 2, 2048, 2048
NH, NKV, HD = 16, 8, 128
SCALE = HD ** -0.5
NC = 8
TT = B * T  # 4096 flat tokens, b-major
NEG = -3.0e38  # large-negative additive mask in the 2^32-scaled score domain

F32 = mybir.dt.float32
BF = mybir.dt.bfloat16
F8 = mybir.dt.float8e4

NPBF = ml_dtypes.bfloat16
NPF8 = ml_dtypes.float8_e4m3

TN = 512           # token chunk for projection moving operand
NG = TT // TN      # 8 projection token groups
NHC = HID // 128   # 16 contraction chunks
NQC = T // 256     # 8 query strips of 256 per batch
NKC = T // 128     # 16 key chunks of 128 per batch

# power-of-two fp8 scaling (descale 2^-32 folds into the Exp input scale)
A_H = 64.0         # hidden activations
A_Q = 4096.0       # q weights (on top of the folded SCALE)
A_K = 256.0        # k weights
EXP_SCALE = 1.0 / (A_H * A_H * A_Q * A_K)  # == 2^-32

_CACHE = {}
last_results = None  # test harness reads exec_time_ns from here


def _build(mode: str):
    """Build the SPMD graph. mode in {"causal", "none", "generic"}."""
    nc = bacc.Bacc("TRN2", target_bir_lowering=False, debug=False, num_devices=NC)

    hT8_e = nc.declare_dram_parameter("hT8", [NG, NHC, 128, TN], F8, isOutput=False)
    hTv_e = nc.declare_dram_parameter("hTv", [NG, NHC, 128, TN], BF, isOutput=False)
    w8_e = nc.declare_dram_parameter("w8", [128, 3, NHC, 128], F8, isOutput=False)
    wv_e = nc.declare_dram_parameter("wv", [128, NHC, 128], BF, isOutput=False)
    woT_e = nc.declare_dram_parameter("woT", [NH * HD, HID], BF, isOutput=False)
    ropeC_e = nc.declare_dram_parameter("ropeC", [128, T], BF, isOutput=False)
    ropeS_e = nc.declare_dram_parameter("ropeS", [128, T], BF, isOutput=False)
    ones_e = nc.declare_dram_parameter("ones", [128, 128], BF, isOutput=False)
    ident_e = nc.declare_dram_parameter("ident", [128, 128], BF, isOutput=False)
    pat_e = None
    maskT_e = None
    if mode == "causal":
        # [key(128), (sub, head, 256q)] additive mask for the two diagonal
        # chunks of a strip, flattened to match the paired score tile layout.
        pat_e = nc.declare_dram_parameter("pat", [128, 1024], F32, isOutput=False)
    elif mode == "generic":
        maskT_e = nc.declare_dram_parameter("maskT", [T, T], F32, isOutput=False)
    out_e = nc.declare_dram_parameter("out", [512, HID], BF, isOutput=True)

    with tile.TileContext(nc) as tc:
        with tc.tile_pool(name="consts", bufs=1) as consts, \
             tc.tile_pool(name="dram", bufs=1, space="DRAM") as dram:

            ones_t = consts.tile([128, 128], BF)
            ident_t = consts.tile([128, 128], BF)
            pat_t = None
            if mode == "causal":
                pat_t = consts.tile([128, 1024], F32)

            a2a_in = [dram.tile([NC, 256, 256], BF, name=f"a2a_in{b}") for b in range(B)]
            a2a_out = [dram.tile([NC, 256, 256], BF, name=f"a2a_out{b}") for b in range(B)]

            # o_proj weights: tiles reserved early (pool-nesting order), DMA
            # emitted mid-stream on the gpsimd queue in 1MB chunks.
            es_wo = ExitStack()
            wop = es_wo.enter_context(tc.tile_pool(name="wop", bufs=1))
            wo_res = [wop.tile([128, NH, 1024], BF, name=f"wo{half}")
                      for half in range(2)]

            es = ExitStack()
            big = es.enter_context(tc.tile_pool(name="big", bufs=1))
            # Persistent activations (my heads, all tokens). q/k are scaled
            # (2^18 / 2^14); v is true-valued.
            q_sb = big.tile([128, 2, TT], BF)      # Q^T, 2 q heads
            k_sb = big.tile([128, TT], BF)         # K^T, 1 kv head
            v_sb = big.tile([128, TT // 128, 128], BF)  # V natural, [tok-chunk, d]

            # -------- Phase A+B interleaved: projection feeds attention ------
            with tc.tile_pool(name="wrope", bufs=1) as wrope, \
                 tc.tile_pool(name="ht8", bufs=8) as ht8_pool, \
                 tc.tile_pool(name="htv", bufs=8) as htv_pool, \
                 tc.tile_pool(name="psA", bufs=2, space="PSUM") as psA, \
                 tc.tile_pool(name="psS", bufs=2, space="PSUM") as psS, \
                 tc.tile_pool(name="ropetmp", bufs=3) as rtmp, \
                 tc.tile_pool(name="vtmp", bufs=2) as vtmp, \
                 tc.tile_pool(name="psPV", bufs=1, space="PSUM") as psPV, \
                 tc.tile_pool(name="psDen", bufs=1, space="PSUM") as psDen, \
                 tc.tile_pool(name="pt", bufs=3) as pt_pool, \
                 tc.tile_pool(name="attev", bufs=2) as attev, \
                 tc.tile_pool(name="mt", bufs=4) as mt_pool:
                ropeC_t = wrope.tile([128, T], BF)
                ropeS_t = wrope.tile([128, T], BF)
                w8_t = wrope.tile([128, 3, NHC, 128], F8)
                wv_t = wrope.tile([128, NHC, 128], BF)
                # Weight/const loads on the scalar HW queue run parallel to
                # the sync-queue activation stream: first matmul needs only
                # w8 stream 0 + the first ht8 unit.
                for s in range(3):
                    nc.scalar.dma_start(w8_t[:, s], w8_e[:, s])
                nc.scalar.dma_start(wv_t[:], wv_e[:])
                nc.scalar.dma_start(ident_t[:], ident_e[:])
                nc.scalar.dma_start(ropeC_t[:], ropeC_e[:])
                nc.scalar.dma_start(ropeS_t[:], ropeS_e[:])
                nc.scalar.dma_start(ones_t[:], ones_e[:])
                if mode == "causal":
                    nc.scalar.dma_start(pat_t[:], pat_e[:])

                def attention_strip(b, qc):
                    cmax = 2 * qc + 2 if mode == "causal" else NKC
                    ntile = cmax // 2
                    mv = q_sb[:, :, b * T + 256 * qc: b * T + 256 * qc + 256]
                    pv = psPV.tile([128, 512], F32, name="pv", tag="pv")
                    den = psDen.tile([128, 512], F32, name="den", tag="den")
                    pts_prev = None
                    den_started = False
                    pt2s = [None] * ntile

                    def emit_pv_den(ti):
                        # PV + denominator for tile ti (software-pipelined: one
                        # tile behind the score matmuls so the Exp latency is
                        # hidden behind the next tile's scores)
                        nonlocal pts_prev, den_started
                        c0 = 2 * ti
                        pt2 = pt2s[ti]
                        last = ti == ntile - 1
                        nc.tensor.matmul(pv[:], v_sb[:, NKC * b + c0, :], pt2[:, 0:512],
                                         start=(ti == 0), stop=False)
                        nc.tensor.matmul(pv[:], v_sb[:, NKC * b + c0 + 1, :], pt2[:, 512:1024],
                                         start=False, stop=last)
                        pts = pt_pool.tile([128, 512], BF, name="pts", tag="pts")
                        nc.vector.tensor_add(pts[:], pt2[:, 0:512], pt2[:, 512:1024])
                        if ti % 2 == 0 and not last:
                            pts_prev = pts
                        else:
                            if ti % 2 == 1:
                                ptm = pt_pool.tile([128, 512], BF, name="ptm", tag="pts")
                                nc.vector.tensor_add(ptm[:], pts_prev[:], pts[:])
                            else:
                                ptm = pts
                            nc.tensor.matmul(den[:], ones_t[:], ptm[:],
                                             start=not den_started, stop=last)
                            den_started = True

                    for ti in range(ntile):
                        c0 = 2 * ti
                        st2 = psS.tile([128, 1024], F32, name="st2", tag="st2")
                        nc.tensor.matmul(
                            st2[:, 0:512],
                            k_sb[:, b * T + 128 * c0: b * T + 128 * c0 + 128],
                            mv, start=True, stop=True)
                        nc.tensor.matmul(
                            st2[:, 512:1024],
                            k_sb[:, b * T + 128 * c0 + 128: b * T + 128 * c0 + 256],
                            mv, start=True, stop=True)
                        if mode == "causal" and ti == ntile - 1:
                            # both diagonal chunks masked by one 1024-wide add
                            nc.vector.tensor_add(st2[:], st2[:], pat_t[:])
                        elif mode == "generic":
                            for half in range(2):
                                mt = mt_pool.tile([128, 256], F32, name="mt", tag="mt")
                                nc.scalar.dma_start(
                                    mt[:], maskT_e[128 * (c0 + half):128 * (c0 + half) + 128,
                                                   256 * qc:256 * qc + 256])
                                o = 512 * half
                                nc.vector.tensor_add(st2[:, o:o + 256], st2[:, o:o + 256], mt[:])
                                nc.vector.tensor_add(st2[:, o + 256:o + 512], st2[:, o + 256:o + 512], mt[:])
                        pt2 = pt_pool.tile([128, 1024], BF, name="pt2", tag="pt2")
                        nc.scalar.activation(pt2[:], st2[:],
                                             mybir.ActivationFunctionType.Exp,
                                             scale=EXP_SCALE)
                        pt2s[ti] = pt2
                        if ti > 0:
                            emit_pv_den(ti - 1)
                    emit_pv_den(ntile - 1)
                    # den rows are all identical (ones stationary) == softmax denom
                    den_rb = attev.tile([128, 512], F32, name="den_rb", tag="den_rb")
                    nc.vector.reciprocal_approx_fast(den_rb[:], den[:])
                    ao = attev.tile([128, 512], BF, name="ao", tag="ao")
                    nc.vector.tensor_mul(ao[:], pv[:], den_rb[:])
                    # ao on gpsimd: on the scalar queue this descriptor (which
                    # waits on the whole strip pipeline) head-of-line blocks
                    # the next strip's exps and delays the AllToAll trigger
                    nc.gpsimd.dma_start(
                        a2a_in[b][qc].rearrange("(h p) t -> p h t", p=128),
                        ao[:].rearrange("p (h t) -> p h t", h=2))

                for g in range(NG):
                    t0 = g * TN
                    # chunk-pair slices for the fp8 DoubleRow moving operand;
                    # group 0 uses small units so the first matmuls start as
                    # early as possible (DMA-paced kernel head)
                    ht8_pairs = []
                    htvs = []
                    if g == 0:
                        for j in range(NHC // 4):
                            ht8 = ht8_pool.tile([128, 4, TN], F8, name="ht8", tag="ht8")
                            for h2 in range(2):
                                nc.sync.dma_start(
                                    ht8[:, 2 * h2:2 * h2 + 2, :],
                                    hT8_e[g, 4 * j + 2 * h2:4 * j + 2 * h2 + 2]
                                    .rearrange("c p t -> p c t"))
                                ht8_pairs.append(ht8[:, 2 * h2:2 * h2 + 2, :])
                    else:
                        for j in range(NHC // 4):
                            ht8 = ht8_pool.tile([128, 4, TN], F8, name="ht8", tag="ht8")
                            nc.sync.dma_start(ht8[:], hT8_e[g, 4 * j:4 * j + 4].rearrange("c p t -> p c t"))
                            ht8_pairs.append(ht8[:, 0:2, :])
                            ht8_pairs.append(ht8[:, 2:4, :])
                    for j in range(NHC // 4):
                        htv = htv_pool.tile([128, 4, TN], BF, name="htv", tag="htv")
                        nc.gpsimd.dma_start(htv[:], hTv_e[g, 4 * j:4 * j + 4].rearrange("c p t -> p c t"))
                        htvs.append(htv)
                    # o_proj weights load in the back half: the kernel front
                    # saturates HBM with the activation stream (measured), the
                    # back half has DMA slack and phase C needs wo only at the
                    # very end.
                    if g in (5, 6):
                        half = g - 5
                        for hco in range(4):
                            nc.scalar.dma_start(
                                wo_res[half][:, 4 * hco:4 * hco + 4, :],
                                woT_e[512 * hco:512 * (hco + 1),
                                      half * 1024:(half + 1) * 1024]
                                .rearrange("(h p) n -> p h n", p=128))
                    ctab = g % (T // TN) * TN  # rope table column offset
                    # q1/q2/k: fp8 DoubleRow, 256-deep contraction per matmul
                    for s in range(3):
                        ps = psA.tile([128, TN], F32, name="psA", tag="psA")
                        for j in range(NHC // 2):
                            nc.tensor.matmul(ps[:], w8_t[:, s, 2 * j:2 * j + 2, :],
                                             ht8_pairs[j],
                                             start=(j == 0), stop=(j == NHC // 2 - 1),
                                             perf_mode=mybir.MatmulPerfMode.DoubleRow)
                        # RoPE: out = ps*C + rot(ps)*S  (S carries the sign)
                        if s < 2:
                            dst = q_sb[:, s, t0:t0 + TN]
                        else:
                            dst = k_sb[:, t0:t0 + TN]
                        csl = ropeC_t[:, ctab:ctab + TN]
                        ssl = ropeS_t[:, ctab:ctab + TN]
                        t1 = rtmp.tile([128, TN], BF, name="t1", tag="t1")
                        t2 = rtmp.tile([128, TN], BF, name="t2", tag="t2")
                        nc.vector.tensor_mul(t1[:], ps[:], csl)
                        nc.vector.tensor_mul(t2[0:64, :], ps[64:128, :], ssl[0:64, :])
                        nc.vector.tensor_mul(t2[64:128, :], ps[0:64, :], ssl[64:128, :])
                        nc.vector.tensor_add(dst, t1[:], t2[:])
                    # v: bf16, V^T -> transpose to V natural via PE
                    psv = psA.tile([128, TN], F32, name="psv", tag="psA")
                    for hc in range(NHC):
                        nc.tensor.matmul(psv[:], wv_t[:, hc, :],
                                         htvs[hc // 4][:, hc % 4, :],
                                         start=(hc == 0), stop=(hc == NHC - 1))
                    vt = vtmp.tile([128, TN], BF, name="vt", tag="vt")
                    nc.scalar.copy(vt[:], psv[:])
                    for jj in range(TN // 128):
                        trp = psS.tile([128, 1024], F32, name="trp", tag="st2")
                        trp_bf = trp.bitcast(BF)[:, 0:128]
                        nc.tensor.transpose(trp_bf, vt[:, jj * 128:(jj + 1) * 128], ident_t[:])
                        if jj % 2 == 0:
                            nc.vector.tensor_copy(v_sb[:, g * (TN // 128) + jj, :], trp_bf)
                        else:
                            nc.scalar.copy(v_sb[:, g * (TN // 128) + jj, :], trp_bf)
                    # attention strips unlocked by this group
                    if mode == "causal":
                        b = g // 4
                        strips = [(b, 2 * (g % 4)), (b, 2 * (g % 4) + 1)]
                    else:
                        # non-causal strips read every key chunk of the batch
                        strips = ([(g // 4, qc) for qc in range(NQC)]
                                  if g in (3, 7) else [])
                    for b, qc in strips:
                        attention_strip(b, qc)
                    if g in (3, 7):
                        nc.gpsimd.collective_compute(
                            "AllToAll", mybir.AluOpType.bypass,
                            replica_groups=[list(range(NC))],
                            ins=[a2a_in[g // 4][:].opt()],
                            outs=[a2a_out[g // 4][:].opt()])

            es.close()  # free q/k/v SBUF before o_proj

            # ---------------- Phase C: o_proj --------------------------------
            with tc.tile_pool(name="attg", bufs=2) as attg_pool, \
                 tc.tile_pool(name="psF", bufs=2, space="PSUM") as psF, \
                 tc.tile_pool(name="fo", bufs=2) as fo_pool:
                for p in range(B):
                    att_g = attg_pool.tile([128, NH, 256], BF, name="attg", tag="attg")
                    for j in range(NC):
                        nc.sync.dma_start(
                            att_g[:, 2 * j:2 * j + 2, :],
                            a2a_out[p][j].rearrange("(h p) t -> p h t", p=128))
                    fins = [psF.tile([128, HID], F32, name="fin", tag="fin") for _ in range(2)]
                    # tch-outer so fins[0] completes early and its copies/DMA
                    # overlap fins[1]'s matmuls (shorter serial tail)
                    for tch in range(2):
                        for half in range(2):
                            for h in range(NH):
                                for n2 in range(2):
                                    nc.tensor.matmul(
                                        fins[tch][:, half * 1024 + n2 * 512: half * 1024 + (n2 + 1) * 512],
                                        att_g[:, h, tch * 128:(tch + 1) * 128],
                                        wo_res[half][:, h, n2 * 512:(n2 + 1) * 512],
                                        start=(h == 0), stop=(h == NH - 1))
                        fo = fo_pool.tile([128, HID], BF, name="fo", tag="fo")
                        for seg in range(4):
                            sl = slice(512 * seg, 512 * (seg + 1))
                            if seg % 2 == 0:
                                nc.vector.tensor_copy(fo[:, sl], fins[tch][:, sl])
                            else:
                                nc.scalar.copy(fo[:, sl], fins[tch][:, sl])
                            if seg % 2 == 1:
                                # write out per-half so the final DMA tail is short
                                nc.sync.dma_start(
                                    out_e[p * 256 + tch * 128: p * 256 + (tch + 1) * 128,
                                          1024 * (seg // 2):1024 * (seg // 2) + 1024],
                                    fo[:, 1024 * (seg // 2):1024 * (seg // 2) + 1024])
            es_wo.close()

    nc.compile()
    return nc


def _host_prep(hidden_states, freqs_cos, freqs_sin, mask, w_qkv, w_o, kv_write_indices):
    idx = np.asarray(kv_write_indices).astype(np.int64)
    if not np.array_equal(idx, np.arange(T, dtype=np.int64)):
        raise NotImplementedError("kernel specialized for kv_write_indices == arange(T)")

    hs = np.asarray(hidden_states, dtype=np.float32).reshape(TT, HID)
    # [HID, TT] -> tiled [NG, NHC, 128, TN] so each DMA slice is contiguous
    hT = np.ascontiguousarray(
        hs.T.reshape(NHC, 128, NG, TN).transpose(2, 0, 1, 3))
    hTv = hT.astype(NPBF)
    hT8 = (hT * A_H).astype(NPF8)

    m2 = np.asarray(mask, dtype=np.float32).reshape(T, T)
    tril = np.tril(np.ones((T, T), dtype=bool))
    if not m2.any():
        mode = "none"
    elif (m2[tril] == 0).all() and (m2[~tril] <= -1e30).all():
        mode = "causal"
    else:
        mode = "generic"

    wq = np.asarray(w_qkv, dtype=np.float32)
    woT = np.ascontiguousarray(np.asarray(w_o, dtype=np.float32).T).astype(NPBF)

    def tile_w(wrows):
        # [128 out, HID] -> [NHC, 128 hid, 128 out] stationary tiles
        return np.ascontiguousarray(wrows.T).reshape(NHC, 128, 128)

    w8s, wvs = [], []
    for c in range(NC):
        q1 = wq[(2 * c) * HD:(2 * c + 1) * HD] * (SCALE * A_Q)
        q2 = wq[(2 * c + 1) * HD:(2 * c + 2) * HD] * (SCALE * A_Q)
        k = wq[NH * HD + c * HD: NH * HD + (c + 1) * HD] * A_K
        v = wq[(NH + NKV) * HD + c * HD: (NH + NKV) * HD + (c + 1) * HD]
        # [3, NHC, 128 hid, 128 out] -> [128 hid, 3, NHC, 128 out] (SBUF layout)
        w8s.append(np.ascontiguousarray(
            np.stack([tile_w(q1), tile_w(q2), tile_w(k)])
            .transpose(2, 0, 1, 3)).astype(NPF8))
        wvs.append(np.ascontiguousarray(
            tile_w(v).transpose(1, 0, 2)).astype(NPBF))

    cosT = np.asarray(freqs_cos, dtype=np.float32).T  # [64, T]
    sinT = np.asarray(freqs_sin, dtype=np.float32).T
    ropeC = np.ascontiguousarray(np.concatenate([cosT, cosT], axis=0)).astype(NPBF)
    ropeS = np.ascontiguousarray(np.concatenate([-sinT, sinT], axis=0)).astype(NPBF)

    consts = {
        "ropeC": ropeC,
        "ropeS": ropeS,
        "ones": np.ones((128, 128), NPBF),
        "ident": np.eye(128, dtype=np.float32).astype(NPBF),
    }
    if mode == "causal":
        # [key 128, (sub, head, 256 q)] for the strip-diagonal chunk pair
        kr = np.arange(128)[:, None]
        qr = np.arange(256)[None, :]
        pats = []
        for sub in range(2):
            p = np.where(kr + 128 * sub <= qr, np.float32(0.0), np.float32(NEG))
            pats.append(np.broadcast_to(p[:, None, :], (128, 2, 256)))
        pat = np.ascontiguousarray(
            np.concatenate(pats, axis=1).reshape(128, 1024)).astype(np.float32)
        consts["pat"] = pat
    elif mode == "generic":
        # mask values live in the 2^32-scaled score domain; clamp so the DVE
        # add cannot overflow fp32
        mscaled = np.maximum(m2.T.astype(np.float64) / EXP_SCALE, NEG)
        consts["maskT"] = np.ascontiguousarray(mscaled.astype(np.float32))

    in_maps = []
    for c in range(NC):
        m = {"hT8": hT8, "hTv": hTv, "w8": w8s[c], "wv": wvs[c], "woT": woT}
        m.update(consts)
        in_maps.append(m)
    return mode, in_maps


def kernel(hidden_states, freqs_cos, freqs_sin, k_cache, v_cache, mask, w_qkv,
           w_o, kv_write_indices):
    # k_cache/v_cache are fully overwritten (kv_write_indices == arange covers
    # every slot), so their incoming contents are irrelevant.
    global last_results
    mode, in_maps = _host_prep(hidden_states, freqs_cos, freqs_sin, mask,
                               w_qkv, w_o, kv_write_indices)
    if mode not in _CACHE:
        _CACHE[mode] = _build(mode)
    nc = _CACHE[mode]

    trace = bool(os.environ.get("BASS_KERNEL_TRACE"))
    res = run_bass_kernel_spmd(nc, in_maps, core_ids=list(range(NC)), trace=trace)
    last_results = res

    final = np.empty((B, T, HID), dtype=np.float32)
    for c in range(NC):
        o = np.asarray(res.results[c]["out"]).astype(np.float32)
        final[0, 256 * c:256 * (c + 1)] = o[0:256]
        final[1, 256 * c:256 * (c + 1)] = o[256:512]
    return final


# revision 26
# speedup vs baseline: 1.0793x; 1.0342x over previous
"""Trainium2 Bass kernel: GQA attention layer (RoPE + causal attention + projections).

Strategy (8 NeuronCores, tensor-parallel by head):
  - Each core owns 2 query heads + 1 kv head (NH=16, NKV=8 -> GQA pairs align
    with cores exactly). QKV projection, RoPE, and attention for those heads run
    fully locally -- zero K/V communication.
  - q1/q2/k projections run in fp8 (e4m3) with DoubleRow perf mode (256-deep
    contraction per instruction, 2 fp8 mults/cell) -- safe because scores are
    tiny and softmax normalization cancels quantization noise. The v projection
    and everything downstream stays bf16 (attention output is a near-uniform
    mean over tokens, so independent operand noise passes through at full
    relative strength -- fp8 there would blow the error budget).
  - Activations/weights are scaled by powers of two host-side so fp8 operands
    sit in e4m3's sweet spot; q/k stay scaled through RoPE and QK^T and the
    combined 2^-32 descale folds into the Exp activation's input scale (exact).
  - Attention is computed in the S^T orientation ([keys, q]); score chunks are
    computed in pairs into a [128,1024] PSUM tile so one Exp activation covers
    two chunks; softmax denominator accumulates via an all-ones stationary
    matmul over 4-chunk partial sums (DVE pre-adds).
  - DMAs are spread over three queues (sync HW, scalar HW, gpsimd SW) so
    weight/const loads run parallel to the activation stream and phase-C input
    loads are not head-of-line blocked behind attention stores.
  - After attention, one AllToAll per batch reshards activations from
    head-sharded to token-sharded; each core then runs o_proj for its 512
    tokens (bf16, fp32 PSUM) and emits bf16 outputs (host casts to fp32).
"""

import os
from contextlib import ExitStack

import ml_dtypes
import numpy as np

import concourse.bass as bass
import concourse.tile as tile
from concourse import bacc, mybir
from concourse.bass_utils import run_bass_kernel_spmd

# Problem shapes (hardcoded per spec nn_AvaAttention_36249523978775).
B, T, HID = 2, 2048, 2048
NH, NKV, HD = 16, 8, 128
SCALE = HD ** -0.5
NC = 8
TT = B * T  # 4096 flat tokens, b-major
NEG = -3.0e38  # large-negative additive mask in the 2^32-scaled score domain

F32 = mybir.dt.float32
BF = mybir.dt.bfloat16
F8 = mybir.dt.float8e4

NPBF = ml_dtypes.bfloat16
NPF8 = ml_dtypes.float8_e4m3

TN = 512           # token chunk for projection moving operand
NG = TT // TN      # 8 projection token groups
NHC = HID // 128   # 16 contraction chunks
NQC = T // 256     # 8 query strips of 256 per batch
NKC = T // 128     # 16 key chunks of 128 per batch

# power-of-two fp8 scaling (descale 2^-32 folds into the Exp input scale)
A_H = 64.0         # hidden activations
A_Q = 4096.0       # q weights (on top of the folded SCALE)
A_K = 256.0        # k weights
EXP_SCALE = 1.0 / (A_H * A_H * A_Q * A_K)  # == 2^-32

_CACHE = {}
last_results = None  # test harness reads exec_time_ns from here


def _build(mode: str):
    """Build the SPMD graph. mode in {"causal", "none", "generic"}."""
    nc = bacc.Bacc("TRN2", target_bir_lowering=False, debug=False, num_devices=NC)

    # activation/weight layouts are partition-major so every DMA moves long
    # contiguous per-partition runs (small packets starve the DMA engines)
    hT8_e = nc.declare_dram_parameter("hT8", [NG, NHC // 4, 128, 4, TN], F8, isOutput=False)
    hTv_e = nc.declare_dram_parameter("hTv", [NG, NHC // 4, 128, 4, TN], BF, isOutput=False)
    w8_e = nc.declare_dram_parameter("w8", [128, 3, NHC, 128], F8, isOutput=False)
    wv_e = nc.declare_dram_parameter("wv", [128, NHC, 128], BF, isOutput=False)
    woT_e = nc.declare_dram_parameter("woT", [2, 4, 128, 4, 1024], BF, isOutput=False)
    ropeC_e = nc.declare_dram_parameter("ropeC", [128, T], BF, isOutput=False)
    ropeS_e = nc.declare_dram_parameter("ropeS", [128, T], BF, isOutput=False)
    ones_e = nc.declare_dram_parameter("ones", [128, 128], BF, isOutput=False)
    ident_e = nc.declare_dram_parameter("ident", [128, 128], BF, isOutput=False)
    pat_e = None
    maskT_e = None
    if mode == "causal":
        # [key(128), (sub, head, 256q)] additive mask for the two diagonal
        # chunks of a strip, flattened to match the paired score tile layout.
        pat_e = nc.declare_dram_parameter("pat", [128, 1024], F32, isOutput=False)
    elif mode == "generic":
        maskT_e = nc.declare_dram_parameter("maskT", [T, T], F32, isOutput=False)
    out_e = nc.declare_dram_parameter("out", [512, HID], BF, isOutput=True)

    with tile.TileContext(nc) as tc:
        with tc.tile_pool(name="consts", bufs=1) as consts, \
             tc.tile_pool(name="dram", bufs=1, space="DRAM") as dram:

            ones_t = consts.tile([128, 128], BF)
            ident_t = consts.tile([128, 128], BF)
            pat_t = None
            if mode == "causal":
                pat_t = consts.tile([128, 1024], F32)

            a2a_in = [dram.tile([NC, 128, 2, 256], BF, name=f"a2a_in{b}") for b in range(B)]
            a2a_out = [dram.tile([NC, 128, 2, 256], BF, name=f"a2a_out{b}") for b in range(B)]

            # o_proj weights: tiles reserved early (pool-nesting order), DMA
            # emitted mid-stream on the gpsimd queue in 1MB chunks.
            es_wo = ExitStack()
            wop = es_wo.enter_context(tc.tile_pool(name="wop", bufs=1))
            wo_res = [wop.tile([128, NH, 1024], BF, name=f"wo{half}")
                      for half in range(2)]

            es = ExitStack()
            big = es.enter_context(tc.tile_pool(name="big", bufs=1))
            # Persistent activations (my heads, all tokens). q/k are scaled
            # (2^18 / 2^14); v is true-valued.
            q_sb = big.tile([128, 2, TT], BF)      # Q^T, 2 q heads
            k_sb = big.tile([128, TT], BF)         # K^T, 1 kv head
            v_sb = big.tile([128, TT // 128, 128], BF)  # V natural, [tok-chunk, d]

            # -------- Phase A+B interleaved: projection feeds attention ------
            with tc.tile_pool(name="wrope", bufs=1) as wrope, \
                 tc.tile_pool(name="ht8", bufs=8) as ht8_pool, \
                 tc.tile_pool(name="htv", bufs=8) as htv_pool, \
                 tc.tile_pool(name="psA", bufs=2, space="PSUM") as psA, \
                 tc.tile_pool(name="psS", bufs=2, space="PSUM") as psS, \
                 tc.tile_pool(name="ropetmp", bufs=3) as rtmp, \
                 tc.tile_pool(name="vtmp", bufs=2) as vtmp, \
                 tc.tile_pool(name="psPV", bufs=1, space="PSUM") as psPV, \
                 tc.tile_pool(name="psDen", bufs=1, space="PSUM") as psDen, \
                 tc.tile_pool(name="pt", bufs=3) as pt_pool, \
                 tc.tile_pool(name="attev", bufs=2) as attev, \
                 tc.tile_pool(name="mt", bufs=4) as mt_pool:
                ropeC_t = wrope.tile([128, T], BF)
                ropeS_t = wrope.tile([128, T], BF)
                w8_t = wrope.tile([128, 3, NHC, 128], F8)
                wv_t = wrope.tile([128, NHC, 128], BF)
                # Weight/const loads on the scalar HW queue run parallel to
                # the sync-queue activation stream. Critical-first order: the
                # first matmul needs w8 stream 0; group 0's RoPE + strips need
                # the first table columns, pat, ones, ident. Bulk table tails
                # come after.
                nc.scalar.dma_start(w8_t[:, 0], w8_e[:, 0])
                nc.scalar.dma_start(ropeC_t[:, 0:TN], ropeC_e[:, 0:TN])
                nc.scalar.dma_start(ropeS_t[:, 0:TN], ropeS_e[:, 0:TN])
                nc.scalar.dma_start(ident_t[:], ident_e[:])
                nc.scalar.dma_start(ones_t[:], ones_e[:])
                if mode == "causal":
                    nc.scalar.dma_start(pat_t[:], pat_e[:])
                for s in range(1, 3):
                    nc.scalar.dma_start(w8_t[:, s], w8_e[:, s])
                nc.scalar.dma_start(wv_t[:], wv_e[:])
                nc.scalar.dma_start(ropeC_t[:, TN:T], ropeC_e[:, TN:T])
                nc.scalar.dma_start(ropeS_t[:, TN:T], ropeS_e[:, TN:T])

                def attention_strip(b, qc):
                    cmax = 2 * qc + 2 if mode == "causal" else NKC
                    ntile = cmax // 2
                    mv = q_sb[:, :, b * T + 256 * qc: b * T + 256 * qc + 256]
                    pv = psPV.tile([128, 512], F32, name="pv", tag="pv")
                    den = psDen.tile([128, 512], F32, name="den", tag="den")
                    pts_prev = None
                    den_started = False
                    pt2s = [None] * ntile

                    def emit_pv_den(ti):
                        # PV + denominator for tile ti (software-pipelined: one
                        # tile behind the score matmuls so the Exp latency is
                        # hidden behind the next tile's scores)
                        nonlocal pts_prev, den_started
                        c0 = 2 * ti
                        pt2 = pt2s[ti]
                        last = ti == ntile - 1
                        nc.tensor.matmul(pv[:], v_sb[:, NKC * b + c0, :], pt2[:, 0:512],
                                         start=(ti == 0), stop=False)
                        nc.tensor.matmul(pv[:], v_sb[:, NKC * b + c0 + 1, :], pt2[:, 512:1024],
                                         start=False, stop=last)
                        pts = pt_pool.tile([128, 512], BF, name="pts", tag="pts")
                        nc.vector.tensor_add(pts[:], pt2[:, 0:512], pt2[:, 512:1024])
                        if ti % 2 == 0 and not last:
                            pts_prev = pts
                        else:
                            if ti % 2 == 1:
                                ptm = pt_pool.tile([128, 512], BF, name="ptm", tag="pts")
                                nc.vector.tensor_add(ptm[:], pts_prev[:], pts[:])
                            else:
                                ptm = pts
                            nc.tensor.matmul(den[:], ones_t[:], ptm[:],
                                             start=not den_started, stop=last)
                            den_started = True

                    for ti in range(ntile):
                        c0 = 2 * ti
                        st2 = psS.tile([128, 1024], F32, name="st2", tag="st2")
                        nc.tensor.matmul(
                            st2[:, 0:512],
                            k_sb[:, b * T + 128 * c0: b * T + 128 * c0 + 128],
                            mv, start=True, stop=True)
                        nc.tensor.matmul(
                            st2[:, 512:1024],
                            k_sb[:, b * T + 128 * c0 + 128: b * T + 128 * c0 + 256],
                            mv, start=True, stop=True)
                        if mode == "causal" and ti == ntile - 1:
                            # both diagonal chunks masked by one 1024-wide add
                            nc.vector.tensor_add(st2[:], st2[:], pat_t[:])
                        elif mode == "generic":
                            for half in range(2):
                                mt = mt_pool.tile([128, 256], F32, name="mt", tag="mt")
                                nc.scalar.dma_start(
                                    mt[:], maskT_e[128 * (c0 + half):128 * (c0 + half) + 128,
                                                   256 * qc:256 * qc + 256])
                                o = 512 * half
                                nc.vector.tensor_add(st2[:, o:o + 256], st2[:, o:o + 256], mt[:])
                                nc.vector.tensor_add(st2[:, o + 256:o + 512], st2[:, o + 256:o + 512], mt[:])
                        pt2 = pt_pool.tile([128, 1024], BF, name="pt2", tag="pt2")
                        nc.scalar.activation(pt2[:], st2[:],
                                             mybir.ActivationFunctionType.Exp,
                                             scale=EXP_SCALE)
                        pt2s[ti] = pt2
                        if ti > 0:
                            emit_pv_den(ti - 1)
                    emit_pv_den(ntile - 1)
                    # den rows are all identical (ones stationary) == softmax denom
                    den_rb = attev.tile([128, 512], F32, name="den_rb", tag="den_rb")
                    nc.vector.reciprocal_approx_fast(den_rb[:], den[:])
                    ao = attev.tile([128, 512], BF, name="ao", tag="ao")
                    nc.vector.tensor_mul(ao[:], pv[:], den_rb[:])
                    # ao on gpsimd: on the scalar queue this descriptor (which
                    # waits on the whole strip pipeline) head-of-line blocks
                    # the next strip's exps and delays the AllToAll trigger
                    nc.gpsimd.dma_start(
                        a2a_in[b][qc],
                        ao[:].rearrange("p (h t) -> p h t", h=2))

                for g in range(NG):
                    t0 = g * TN
                    # chunk-pair slices for the fp8 DoubleRow moving operand;
                    # group 0 uses small units so the first matmuls start as
                    # early as possible (DMA-paced kernel head)
                    ht8_pairs = []
                    htvs = []
                    if g == 0:
                        for j in range(NHC // 4):
                            ht8 = ht8_pool.tile([128, 4, TN], F8, name="ht8", tag="ht8")
                            for h2 in range(2):
                                nc.sync.dma_start(
                                    ht8[:, 2 * h2:2 * h2 + 2, :],
                                    hT8_e[g, j][:, 2 * h2:2 * h2 + 2, :])
                                ht8_pairs.append(ht8[:, 2 * h2:2 * h2 + 2, :])
                    else:
                        for j in range(NHC // 4):
                            ht8 = ht8_pool.tile([128, 4, TN], F8, name="ht8", tag="ht8")
                            nc.sync.dma_start(ht8[:], hT8_e[g, j])
                            ht8_pairs.append(ht8[:, 0:2, :])
                            ht8_pairs.append(ht8[:, 2:4, :])
                    for j in range(NHC // 4):
                        htv = htv_pool.tile([128, 4, TN], BF, name="htv", tag="htv")
                        nc.gpsimd.dma_start(htv[:], hTv_e[g, j])
                        htvs.append(htv)
                    # o_proj weights load in the back half: the kernel front
                    # saturates HBM with the activation stream (measured), the
                    # back half has DMA slack and phase C needs wo only at the
                    # very end.
                    if g in (5, 6):
                        half = g - 5
                        for hco in range(4):
                            nc.scalar.dma_start(
                                wo_res[half][:, 4 * hco:4 * hco + 4, :],
                                woT_e[half, hco])
                    ctab = g % (T // TN) * TN  # rope table column offset
                    # q1/q2/k: fp8 DoubleRow, 256-deep contraction per matmul
                    for s in range(3):
                        ps = psA.tile([128, TN], F32, name="psA", tag="psA")
                        for j in range(NHC // 2):
                            nc.tensor.matmul(ps[:], w8_t[:, s, 2 * j:2 * j + 2, :],
                                             ht8_pairs[j],
                                             start=(j == 0), stop=(j == NHC // 2 - 1),
                                             perf_mode=mybir.MatmulPerfMode.DoubleRow)
                        # RoPE: out = ps*C + rot(ps)*S  (S carries the sign)
                        if s < 2:
                            dst = q_sb[:, s, t0:t0 + TN]
                        else:
                            dst = k_sb[:, t0:t0 + TN]
                        csl = ropeC_t[:, ctab:ctab + TN]
                        ssl = ropeS_t[:, ctab:ctab + TN]
                        t1 = rtmp.tile([128, TN], BF, name="t1", tag="t1")
                        t2 = rtmp.tile([128, TN], BF, name="t2", tag="t2")
                        nc.vector.tensor_mul(t1[:], ps[:], csl)
                        nc.vector.tensor_mul(t2[0:64, :], ps[64:128, :], ssl[0:64, :])
                        nc.vector.tensor_mul(t2[64:128, :], ps[0:64, :], ssl[64:128, :])
                        nc.vector.tensor_add(dst, t1[:], t2[:])
                    # v: bf16, V^T -> transpose to V natural via PE
                    psv = psA.tile([128, TN], F32, name="psv", tag="psA")
                    for hc in range(NHC):
                        nc.tensor.matmul(psv[:], wv_t[:, hc, :],
                                         htvs[hc // 4][:, hc % 4, :],
                                         start=(hc == 0), stop=(hc == NHC - 1))
                    vt = vtmp.tile([128, TN], BF, name="vt", tag="vt")
                    nc.scalar.copy(vt[:], psv[:])
                    for jj in range(TN // 128):
                        trp = psS.tile([128, 1024], F32, name="trp", tag="st2")
                        trp_bf = trp.bitcast(BF)[:, 0:128]
                        nc.tensor.transpose(trp_bf, vt[:, jj * 128:(jj + 1) * 128], ident_t[:])
                        if jj % 2 == 0:
                            nc.vector.tensor_copy(v_sb[:, g * (TN // 128) + jj, :], trp_bf)
                        else:
                            nc.scalar.copy(v_sb[:, g * (TN // 128) + jj, :], trp_bf)
                    # attention strips unlocked by this group
                    if mode == "causal":
                        b = g // 4
                        strips = [(b, 2 * (g % 4)), (b, 2 * (g % 4) + 1)]
                    else:
                        # non-causal strips read every key chunk of the batch
                        strips = ([(g // 4, qc) for qc in range(NQC)]
                                  if g in (3, 7) else [])
                    for b, qc in strips:
                        attention_strip(b, qc)
                    if g in (3, 7):
                        nc.gpsimd.collective_compute(
                            "AllToAll", mybir.AluOpType.bypass,
                            replica_groups=[list(range(NC))],
                            ins=[a2a_in[g // 4][:].opt()],
                            outs=[a2a_out[g // 4][:].opt()])

            es.close()  # free q/k/v SBUF before o_proj

            # ---------------- Phase C: o_proj --------------------------------
            with tc.tile_pool(name="attg", bufs=2) as attg_pool, \
                 tc.tile_pool(name="psF", bufs=2, space="PSUM") as psF, \
                 tc.tile_pool(name="fo", bufs=2) as fo_pool:
                for p in range(B):
                    att_g = attg_pool.tile([128, NH, 256], BF, name="attg", tag="attg")
                    for j in range(NC):
                        nc.sync.dma_start(att_g[:, 2 * j:2 * j + 2, :], a2a_out[p][j])
                    fins = [psF.tile([128, HID], F32, name="fin", tag="fin") for _ in range(2)]
                    # tch-outer so fins[0] completes early and its copies/DMA
                    # overlap fins[1]'s matmuls (shorter serial tail)
                    for tch in range(2):
                        for half in range(2):
                            for h in range(NH):
                                for n2 in range(2):
                                    nc.tensor.matmul(
                                        fins[tch][:, half * 1024 + n2 * 512: half * 1024 + (n2 + 1) * 512],
                                        att_g[:, h, tch * 128:(tch + 1) * 128],
                                        wo_res[half][:, h, n2 * 512:(n2 + 1) * 512],
                                        start=(h == 0), stop=(h == NH - 1))
                        fo = fo_pool.tile([128, HID], BF, name="fo", tag="fo")
                        for seg in range(4):
                            sl = slice(512 * seg, 512 * (seg + 1))
                            if seg % 2 == 0:
                                nc.vector.tensor_copy(fo[:, sl], fins[tch][:, sl])
                            else:
                                nc.scalar.copy(fo[:, sl], fins[tch][:, sl])
                            if seg % 2 == 1:
                                # write out per-half so the final DMA tail is short
                                nc.sync.dma_start(
                                    out_e[p * 256 + tch * 128: p * 256 + (tch + 1) * 128,
                                          1024 * (seg // 2):1024 * (seg // 2) + 1024],
                                    fo[:, 1024 * (seg // 2):1024 * (seg // 2) + 1024])
            es_wo.close()

    nc.compile()
    return nc


def _host_prep(hidden_states, freqs_cos, freqs_sin, mask, w_qkv, w_o, kv_write_indices):
    idx = np.asarray(kv_write_indices).astype(np.int64)
    if not np.array_equal(idx, np.arange(T, dtype=np.int64)):
        raise NotImplementedError("kernel specialized for kv_write_indices == arange(T)")

    hs = np.asarray(hidden_states, dtype=np.float32).reshape(TT, HID)
    # [HID, TT] -> [NG, NHC/4, 128, 4, TN]: partition-major so each DMA moves
    # one long contiguous run per partition (large packets keep the DMA
    # engines at full rate)
    hT = hs.T.reshape(NHC, 128, NG, TN).transpose(2, 0, 1, 3)  # [NG, NHC, 128, TN]
    hT = np.ascontiguousarray(
        hT.reshape(NG, NHC // 4, 4, 128, TN).transpose(0, 1, 3, 2, 4))
    hTv = hT.astype(NPBF)
    hT8 = (hT * A_H).astype(NPF8)

    m2 = np.asarray(mask, dtype=np.float32).reshape(T, T)
    tril = np.tril(np.ones((T, T), dtype=bool))
    if not m2.any():
        mode = "none"
    elif (m2[tril] == 0).all() and (m2[~tril] <= -1e30).all():
        mode = "causal"
    else:
        mode = "generic"

    wq = np.asarray(w_qkv, dtype=np.float32)
    # w_o^T [2048=(hco,h,p), 2048=(half,n)] -> [half, hco, 128p, 4h, 1024n]
    woT = np.ascontiguousarray(
        np.asarray(w_o, dtype=np.float32).T
        .reshape(4, 4, 128, 2, 1024).transpose(3, 0, 2, 1, 4)).astype(NPBF)

    def tile_w(wrows):
        # [128 out, HID] -> [NHC, 128 hid, 128 out] stationary tiles
        return np.ascontiguousarray(wrows.T).reshape(NHC, 128, 128)

    w8s, wvs = [], []
    for c in range(NC):
        q1 = wq[(2 * c) * HD:(2 * c + 1) * HD] * (SCALE * A_Q)
        q2 = wq[(2 * c + 1) * HD:(2 * c + 2) * HD] * (SCALE * A_Q)
        k = wq[NH * HD + c * HD: NH * HD + (c + 1) * HD] * A_K
        v = wq[(NH + NKV) * HD + c * HD: (NH + NKV) * HD + (c + 1) * HD]
        # [3, NHC, 128 hid, 128 out] -> [128 hid, 3, NHC, 128 out] (SBUF layout)
        w8s.append(np.ascontiguousarray(
            np.stack([tile_w(q1), tile_w(q2), tile_w(k)])
            .transpose(2, 0, 1, 3)).astype(NPF8))
        wvs.append(np.ascontiguousarray(
            tile_w(v).transpose(1, 0, 2)).astype(NPBF))

    cosT = np.asarray(freqs_cos, dtype=np.float32).T  # [64, T]
    sinT = np.asarray(freqs_sin, dtype=np.float32).T
    ropeC = np.ascontiguousarray(np.concatenate([cosT, cosT], axis=0)).astype(NPBF)
    ropeS = np.ascontiguousarray(np.concatenate([-sinT, sinT], axis=0)).astype(NPBF)

    consts = {
        "ropeC": ropeC,
        "ropeS": ropeS,
        "ones": np.ones((128, 128), NPBF),
        "ident": np.eye(128, dtype=np.float32).astype(NPBF),
    }
    if mode == "causal":
        # [key 128, (sub, head, 256 q)] for the strip-diagonal chunk pair
        kr = np.arange(128)[:, None]
        qr = np.arange(256)[None, :]
        pats = []
        for sub in range(2):
            p = np.where(kr + 128 * sub <= qr, np.float32(0.0), np.float32(NEG))
            pats.append(np.broadcast_to(p[:, None, :], (128, 2, 256)))
        pat = np.ascontiguousarray(
            np.concatenate(pats, axis=1).reshape(128, 1024)).astype(np.float32)
        consts["pat"] = pat
    elif mode == "generic":
        # mask values live in the 2^32-scaled score domain; clamp so the DVE
        # add cannot overflow fp32
        mscaled = np.maximum(m2.T.astype(np.float64) / EXP_SCALE, NEG)
        consts["maskT"] = np.ascontiguousarray(mscaled.astype(np.float32))

    in_maps = []
    for c in range(NC):
        m = {"hT8": hT8, "hTv": hTv, "w8": w8s[c], "wv": wvs[c], "woT": woT}
        m.update(consts)
        in_maps.append(m)
    return mode, in_maps


def kernel(hidden_states, freqs_cos, freqs_sin, k_cache, v_cache, mask, w_qkv,
           w_o, kv_write_indices):
    # k_cache/v_cache are fully overwritten (kv_write_indices == arange covers
    # every slot), so their incoming contents are irrelevant.
    global last_results
    mode, in_maps = _host_prep(hidden_states, freqs_cos, freqs_sin, mask,
                               w_qkv, w_o, kv_write_indices)
    if mode not in _CACHE:
        _CACHE[mode] = _build(mode)
    nc = _CACHE[mode]

    trace = bool(os.environ.get("BASS_KERNEL_TRACE"))
    res = run_bass_kernel_spmd(nc, in_maps, core_ids=list(range(NC)), trace=trace)
    last_results = res

    final = np.empty((B, T, HID), dtype=np.float32)
    for c in range(NC):
        o = np.asarray(res.results[c]["out"]).astype(np.float32)
        final[0, 256 * c:256 * (c + 1)] = o[0:256]
        final[1, 256 * c:256 * (c + 1)] = o[256:512]
    return final
